# revision 39
# baseline (speedup 1.0000x reference)
"""Nearest-class-mean softmax scores on 8 Trainium2 NeuronCores.

Computes softmax(-(||x||^2 + ||mu||^2 - 2 x.mu)) row-wise for
X:[32768,512], muK:[2048,512], with classes where cK==0 masked to the
per-row min score minus 1 before the softmax.

Key facts exploited:
  * softmax is invariant to per-row additive shifts, so the ||x||^2 term
    is dropped, as is any global constant from ||mu||^2 (centered).
  * masked classes (cK==0) have reference probability exp(min-1-max)/Z
    which underflows to exactly 0.0 in fp32 (row score spread ~300 >> 87).
    They are COMPACTED AWAY on the host: the device computes scores only
    for kept classes (padded to C_PAD columns) and the host scatter-fills
    the full [N, 2048] output with zeros elsewhere. This cuts PE matmul
    work, vector work, and output DMA by ~1/3 each, and the lighter
    engine load also lifts the PE out of power throttling (~260 -> ~200
    ns per 512-wide matmul).

Device pipeline per 128-row query tile (data-parallel over rows,
muK replicated; engines balanced so the PE is the only near-saturated
unit):
  psum[128,C_PAD] = (X_tile.T).T @ (2*muK'.T)     PE, fp16 inputs
  sco, mx  = (psum - m2c_bc), rowmax              DVE custom fused op
  nm       = -mx                                  DVE
  ot, Z    = exp(sco + nm), row-sum accum         ACT
  ot      /= Z                                    GpSimd normalize_recip
  out DMA                                         sync-queue issue

The last tile runs its matmuls column-chunk-major with a per-chunk
sub+max so only the final 368-wide chunk's reduction + exp + normalize
remain after the last matmul (drain latency). Input DMAs are ordered so
the first matmul gates on ~160KB.

Known TRN2 runtime pitfalls (bisected on HW, each hangs the device
despite compiling and passing CoreSim): nc.vector.tensor_tensor_reduce,
and dma_start issued from the scalar/ACT queue. nc.gpsimd.normalize_recip
requires load_library(library_config.attn) first.
"""

import numpy as np

import concourse.bass as bass
import concourse.tile as tile
from concourse import bacc, library_config, mybir
from concourse import dve_ops
from concourse.bass_utils import run_bass_kernel_spmd
from concourse.dve_spec import Spec, Src0, Src1, maxx


def _register_sub_max():
    """Custom DVE op: out = in0 - in1 (elementwise), accum_out = rowmax(out)."""
    name = "NCM_SUB_MAX"
    for op in dve_ops.OPS:
        if op.name == name:
            return op

    def _ref(in0, in1, c0, c1, c2):
        b = in0.astype(np.float32) - in1.astype(np.float32)
        return b, b.reshape(b.shape[0], -1).max(axis=-1, keepdims=True)

    spec = Spec(body=Src0 - Src1, accum=maxx, reference=_ref)
    op = dve_ops.DveOp(name, spec, subdim=False, uops_sha={})
    dve_ops._SUB_OPCODE_FOR_NAME[name] = (
        max(dve_ops._SUB_OPCODE_FOR_NAME.values()) + 1)
    assert dve_ops._SUB_OPCODE_FOR_NAME[name] < 0x20
    for ver in ("v3",):
        try:
            op.compile(ver)
        except ValueError as e:  # message carries the freshly-computed sha
            import re
            m = re.search(r"\bv\d+: ([0-9a-f]{16})", str(e))
            op.uops_sha[ver] = m.group(1)
            op.compile(ver)
    dve_ops.OPS.append(op)
    dve_ops.CUSTOM_DVE_SPECS[name] = spec
    return op


NCM_SUB_MAX = _register_sub_max()

N, C, D = 32768, 2048, 512
NCORES = 8
NS = N // NCORES          # 4096 query rows per core
P = 128                   # partitions
KCH = D // P              # 4 contraction chunks of 128
NB = 512                  # matmul moving free-dim (one PSUM bank)
MM_DT = mybir.dt.float16  # matmul operand dtype (1 cycle/row on PE)
F32 = mybir.dt.float32
MASK_M2 = 50000.0         # m2 for padded columns -> score -50000 -> exp==0.0f
C_PAD = 1392              # kept-class columns incl. padding (seed-0 nk=1384)
TILES_PER_BLK = 8         # query tiles per xt DMA block (startup latency)
PSUM_BANK_F32 = 512


def _col_chunks(c_pad):
    return [(c0, min(NB, c_pad - c0)) for c0 in range(0, c_pad, NB)]


def build_nc(c_pad, ns: int = NS):
    """Build the per-core Bass program (SPMD: same program, per-core inputs)."""
    ntiles = ns // P
    nblk = max(1, ntiles // TILES_PER_BLK)
    blkw = ns // nblk  # columns of Xt per block
    tpb = blkw // P    # tiles per block
    chunks = _col_chunks(c_pad)
    psum_w = -(-c_pad // PSUM_BANK_F32) * PSUM_BANK_F32

    nc = bacc.Bacc("TRN2", target_bir_lowering=False)
    xt = nc.dram_tensor("xt", [nblk, KCH, P, blkw], MM_DT, kind="ExternalInput")
    rhs = nc.dram_tensor("rhs", [KCH, P, c_pad], MM_DT, kind="ExternalInput")
    m2bc = nc.dram_tensor("m2bc", [P, c_pad], F32, kind="ExternalInput")
    out = nc.dram_tensor("out", [ns, c_pad], F32, kind="ExternalOutput")

    AF = mybir.ActivationFunctionType
    with tile.TileContext(nc) as tc:
        with (
            tc.tile_pool(name="const", bufs=1) as const,
            tc.tile_pool(name="psum", bufs=2, space=bass.MemorySpace.PSUM) as psum,
            tc.tile_pool(name="psl", bufs=2, space=bass.MemorySpace.PSUM) as psl,
            tc.tile_pool(name="ss", bufs=3) as ssp,
            tc.tile_pool(name="outp", bufs=4) as outp,
            tc.tile_pool(name="stat", bufs=12) as stat,
        ):
            # normalize_recip lives in the attn GPSIMD library; emit the
            # load before any other gpsimd-queue instruction
            nc.gpsimd.load_library(library_config.attn)

            xt_sb = [[const.tile([P, blkw], MM_DT, name=f"xt{b}_{k}")
                      for k in range(KCH)] for b in range(nblk)]
            rhs_sb = [const.tile([P, c_pad], MM_DT, name=f"rhs{k}")
                      for k in range(KCH)]
            m2bc_sb = const.tile([P, c_pad], F32, name="m2bc_sb")

            # startup-latency-aware order: tile 0 only needs the first P
            # columns of each xt[0][k] plus the rhs chunks, in k order; the
            # first matmul gates on ~160KB. The DMA hardware drains one
            # FIFO, so order by first use: block-0 remainders (tiles 1-7)
            # before m2bc (first needed by tile 0's sub+max, ~2.4us after
            # its matmuls start); blocks 2-3 are issued inside the tile
            # loop so early output transfers aren't queued behind them.
            nc.sync.dma_start(rhs_sb[0][:, 0:NB], rhs[0][:, 0:NB])
            nc.sync.dma_start(xt_sb[0][0][:, 0:P], xt[0, 0][:, 0:P])
            nc.sync.dma_start(rhs_sb[0][:, NB:], rhs[0][:, NB:])
            nc.sync.dma_start(xt_sb[0][1][:, 0:P], xt[0, 1][:, 0:P])
            nc.sync.dma_start(rhs_sb[1][:], rhs[1])
            nc.sync.dma_start(xt_sb[0][2][:, 0:P], xt[0, 2][:, 0:P])
            nc.sync.dma_start(rhs_sb[2][:], rhs[2])
            nc.sync.dma_start(xt_sb[0][3][:, 0:P], xt[0, 3][:, 0:P])
            nc.sync.dma_start(rhs_sb[3][:], rhs[3])
            nc.sync.dma_start(xt_sb[0][0][:, P:], xt[0, 0][:, P:])
            nc.sync.dma_start(xt_sb[0][1][:, P:], xt[0, 1][:, P:])
            nc.sync.dma_start(m2bc_sb[:], m2bc[:])
            nc.sync.dma_start(xt_sb[0][2][:, P:], xt[0, 2][:, P:])
            nc.sync.dma_start(xt_sb[0][3][:, P:], xt[0, 3][:, P:])
            for k in range(KCH):
                nc.sync.dma_start(xt_sb[1][k][:], xt[1, k])

            for i in range(ntiles):
                # late xt blocks: enqueue behind the first tiles' output
                # transfers (needed at tiles 16 and 24 — plenty of slack)
                if i == 2 and nblk > 2:
                    for k in range(KCH):
                        nc.sync.dma_start(xt_sb[2][k][:], xt[2, k])
                if i == 5 and nblk > 3:
                    for k in range(KCH):
                        nc.sync.dma_start(xt_sb[3][k][:], xt[3, k])
                last = (i == ntiles - 1)
                ps = psum.tile([P, psum_w], F32)
                blk, off = divmod(i, tpb)
                lhsT = [xt_sb[blk][k][:, off * P:(off + 1) * P]
                        for k in range(KCH)]
                sco = ssp.tile([P, c_pad], F32)
                ot = outp.tile([P, c_pad], F32)
                zs = stat.tile([P, 1], F32)
                nm = stat.tile([P, 1], F32)
                if not last:
                    for k in range(KCH):
                        for c0, w in chunks:
                            nc.tensor.matmul(
                                ps[:, c0:c0 + w], lhsT[k],
                                rhs_sb[k][:, c0:c0 + w],
                                start=(k == 0), stop=(k == KCH - 1),
                            )
                    # sco = 2 x.mu - m2c = scores; mx = rowmax (one DVE pass)
                    mx = stat.tile([P, 1], F32)
                    nc.vector._custom_dve(
                        NCM_SUB_MAX, out=sco[:], accum_out=mx[:],
                        in0=ps[:, 0:c_pad], in1=m2bc_sb[:],
                    )
                    nc.vector.tensor_scalar_mul(nm[:], mx[:], -1.0)
                    # ot = exp(sco - max); zs = sum(ot)
                    nc.scalar.activation(
                        ot[:], sco[:], AF.Exp,
                        bias=nm[:], accum_out=zs[:],
                    )
                    # normalize entirely on the (otherwise idle) GpSimd;
                    # normalize_recip divides by raw zs and clobbers it
                    # with 1/zs, which we don't reuse
                    nc.gpsimd.normalize_recip(ot[:], ot[:], zs[:])
                    nc.sync.dma_start(out[i * P:(i + 1) * P, :], ot[:])
                else:
                    # drain-optimized last tile: c-major matmuls so each
                    # column chunk's sub+max runs under the next chunk's
                    # matmuls; only the final chunk's reduction remains
                    # after the last matmul. Each chunk gets its own
                    # 1-bank psum tile so a chunk's matmuls never wait on
                    # the previous chunk's DVE read (write-after-read)
                    mxs = stat.tile([P, len(chunks)], F32)
                    for ci, (c0, w) in enumerate(chunks):
                        psc = psl.tile([P, NB], F32, name="psc")
                        for k in range(KCH):
                            nc.tensor.matmul(
                                psc[:, 0:w], lhsT[k],
                                rhs_sb[k][:, c0:c0 + w],
                                start=(k == 0), stop=(k == KCH - 1),
                            )
                        nc.vector._custom_dve(
                            NCM_SUB_MAX, out=sco[:, c0:c0 + w],
                            accum_out=mxs[:, ci:ci + 1],
                            in0=psc[:, 0:w],
                            in1=m2bc_sb[:, c0:c0 + w],
                        )
                    nc.vector.tensor_reduce(
                        nm[:], mxs[:], mybir.AxisListType.X,
                        mybir.AluOpType.max, negate=True,
                    )
                    nc.scalar.activation(
                        ot[:], sco[:], AF.Exp,
                        bias=nm[:], accum_out=zs[:],
                    )
                    # normalize+ship in halves: DVE and GpSimd in parallel
                    h = c_pad // 2
                    rz = stat.tile([P, 1], F32)
                    nc.vector.reciprocal(rz[:], zs[:])
                    nc.vector.tensor_scalar_mul(ot[:, :h], ot[:, :h], rz[:])
                    nc.gpsimd.normalize_recip(ot[:, h:], ot[:, h:], zs[:])
                    nc.sync.dma_start(out[i * P:(i + 1) * P, :h], ot[:, :h])
                    nc.sync.dma_start(out[i * P:(i + 1) * P, h:], ot[:, h:])

    nc.compile()
    return nc


_NC_CACHE = {}


def _get_nc(c_pad):
    if c_pad not in _NC_CACHE:
        _NC_CACHE[c_pad] = build_nc(c_pad)
    return _NC_CACHE[c_pad]


def prep_inputs(X, muK, cK, c_pad, keep):
    """Host-side shard/layout prep (numpy only)."""
    X = np.asarray(X, dtype=np.float32)
    muK = np.asarray(muK, dtype=np.float32)
    nk = len(keep)

    muKk = muK[keep]                                    # [nk, D]
    m2 = np.sum(muKk.astype(np.float64) ** 2, axis=1)
    m2c = m2 - m2.mean()  # centered: softmax-invariant shift
    m2p = np.full(c_pad, MASK_M2, dtype=np.float32)
    m2p[:nk] = m2c.astype(np.float32)
    m2bc_np = np.ascontiguousarray(np.broadcast_to(m2p[None, :], (P, c_pad)))

    rhsw = np.zeros((D, c_pad), dtype=np.float16)
    rhsw[:, :nk] = (2.0 * muKk.T).astype(np.float16)
    rhs_np = np.ascontiguousarray(rhsw.reshape(KCH, P, c_pad))

    Xt = X.T.astype(np.float16)  # [D, N]
    ntiles = NS // P
    nblk = max(1, ntiles // TILES_PER_BLK)
    blkw = NS // nblk
    in_maps = []
    for core in range(NCORES):
        xs = Xt[:, core * NS:(core + 1) * NS]              # [D, NS]
        xs = xs.reshape(KCH, P, nblk, blkw).transpose(2, 0, 1, 3)
        in_maps.append({"xt": np.ascontiguousarray(xs),
                        "rhs": rhs_np, "m2bc": m2bc_np})
    return in_maps


def run(X, muK, cK, trace=False, **kw):
    cK = np.asarray(cK, dtype=np.float32)
    keep = np.nonzero(cK != 0.0)[0]
    nk = len(keep)
    if nk == 0:
        # every class masked -> all scores equal -> uniform softmax
        return np.full((N, C), 1.0 / C, dtype=np.float32), None
    c_pad = C_PAD if nk <= C_PAD else C
    in_maps = prep_inputs(X, muK, cK, c_pad, keep)
    nc = _get_nc(c_pad)
    res = run_bass_kernel_spmd(
        nc, in_maps, list(range(NCORES)), trace=trace, **kw)
    compact = np.concatenate(
        [res.results[c]["out"] for c in range(NCORES)], axis=0)
    full = np.zeros((N, C), dtype=np.float32)
    full[:, keep] = compact[:, :nk]
    return full, res


def kernel(X, muK, cK):
    full, _ = run(X, muK, cK, trace=False)
    return full


# revision 43
# speedup vs baseline: 1.1510x; 1.1510x over previous
"""Nearest-class-mean softmax scores on 8 Trainium2 NeuronCores.

Computes softmax(-(||x||^2 + ||mu||^2 - 2 x.mu)) row-wise for
X:[32768,512], muK:[2048,512], with classes where cK==0 masked to the
per-row min score minus 1 before the softmax.

Key facts exploited:
  * softmax is invariant to per-row additive shifts, so the ||x||^2 term
    is dropped, as is any global constant from ||mu||^2 (centered).
  * masked classes (cK==0) have reference probability exp(min-1-max)/Z
    which underflows to exactly 0.0 in fp32 (row score spread ~300 >> 87).
    They are COMPACTED AWAY on the host: the device computes scores only
    for kept classes (padded to C_PAD columns) and the host scatter-fills
    the full [N, 2048] output with zeros elsewhere. This cuts PE matmul
    work, vector work, and output DMA by ~1/3 each, and the lighter
    engine load also lifts the PE out of power throttling (~260 -> ~200
    ns per 512-wide matmul).

Device pipeline per 128-row query tile (data-parallel over rows, muK
replicated; engines balanced so the PE is the only near-saturated unit):
  psum[128,C_PAD] = (X_tile.T).T @ (2*muK'.T)     PE, fp16 inputs
  sco, mx  = (psum - m2c_bc), rowmax              DVE custom fused op
  nm       = -mx                                  DVE
  ot, Z    = exp(sco + nm), row-sum accum         ACT
  ot      /= Z                                    GpSimd normalize_recip
  out DMA                                         sync-queue issue

The last tile runs its matmuls column-chunk-major with a per-chunk
sub+max so only the final chunk's reduction + exp + normalize remain
after the last matmul. Input DMAs are ordered so the first matmul gates
on ~160KB.

Known TRN2 runtime pitfalls (bisected on HW; each compiled and passed
CoreSim, then hung the device): nc.vector.tensor_tensor_reduce, and
dma_start issued from the scalar/ACT queue. nc.gpsimd.normalize_recip
requires load_library(library_config.attn) first. Splitting the last
tile across a second PSUM pool (all 8 banks in use) consistently
regressed the whole-kernel clock by ~13% — leave 2 banks free.
"""

import numpy as np

import concourse.bass as bass
import concourse.tile as tile
from concourse import bacc, library_config, mybir
from concourse import dve_ops
from concourse.bass_utils import run_bass_kernel_spmd
from concourse.dve_spec import Spec, Src0, Src1, maxx


def _register_sub_max():
    """Custom DVE op: out = in0 - in1 (elementwise), accum_out = rowmax(out)."""
    name = "NCM_SUB_MAX"
    for op in dve_ops.OPS:
        if op.name == name:
            return op

    def _ref(in0, in1, c0, c1, c2):
        b = in0.astype(np.float32) - in1.astype(np.float32)
        return b, b.reshape(b.shape[0], -1).max(axis=-1, keepdims=True)

    spec = Spec(body=Src0 - Src1, accum=maxx, reference=_ref)
    op = dve_ops.DveOp(name, spec, subdim=False, uops_sha={})
    dve_ops._SUB_OPCODE_FOR_NAME[name] = (
        max(dve_ops._SUB_OPCODE_FOR_NAME.values()) + 1)
    assert dve_ops._SUB_OPCODE_FOR_NAME[name] < 0x20
    for ver in ("v3",):
        try:
            op.compile(ver)
        except ValueError as e:  # message carries the freshly-computed sha
            import re
            m = re.search(r"\bv\d+: ([0-9a-f]{16})", str(e))
            op.uops_sha[ver] = m.group(1)
            op.compile(ver)
    dve_ops.OPS.append(op)
    dve_ops.CUSTOM_DVE_SPECS[name] = spec
    return op


NCM_SUB_MAX = _register_sub_max()

N, C, D = 32768, 2048, 512
NCORES = 8
NS = N // NCORES          # 4096 query rows per core
P = 128                   # partitions
KCH = D // P              # 4 contraction chunks of 128
NB = 512                  # matmul moving free-dim (one PSUM bank)
MM_DT = mybir.dt.float16  # matmul operand dtype (1 cycle/row on PE)
F32 = mybir.dt.float32
MASK_M2 = 50000.0         # m2 for padded columns -> score -50000 -> exp==0.0f
C_PAD = 1392              # kept-class columns incl. padding (seed-0 nk=1384)
TILES_PER_BLK = 8         # query tiles per xt DMA block (startup latency)
PSUM_BANK_F32 = 512


def _col_chunks(c_pad):
    return [(c0, min(NB, c_pad - c0)) for c0 in range(0, c_pad, NB)]


def build_nc(c_pad, ns: int = NS):
    """Build the per-core Bass program (SPMD: same program, per-core inputs)."""
    ntiles = ns // P
    nblk = max(1, ntiles // TILES_PER_BLK)
    blkw = ns // nblk  # columns of Xt per block
    tpb = blkw // P    # tiles per block
    chunks = _col_chunks(c_pad)
    psum_w = -(-c_pad // PSUM_BANK_F32) * PSUM_BANK_F32

    nc = bacc.Bacc("TRN2", target_bir_lowering=False)
    xt = nc.dram_tensor("xt", [nblk, KCH, P, blkw], MM_DT, kind="ExternalInput")
    rhs = nc.dram_tensor("rhs", [KCH, P, c_pad], MM_DT, kind="ExternalInput")
    m2bc = nc.dram_tensor("m2bc", [P, c_pad], F32, kind="ExternalInput")
    out = nc.dram_tensor("out", [ns, c_pad], F32, kind="ExternalOutput")

    AF = mybir.ActivationFunctionType
    with tile.TileContext(nc) as tc:
        with (
            tc.tile_pool(name="const", bufs=1) as const,
            tc.tile_pool(name="psum", bufs=2, space=bass.MemorySpace.PSUM) as psum,
            tc.tile_pool(name="ss", bufs=3) as ssp,
            tc.tile_pool(name="outp", bufs=4) as outp,
            tc.tile_pool(name="stat", bufs=12) as stat,
        ):
            # normalize_recip lives in the attn GPSIMD library; emit the
            # load before any other gpsimd-queue instruction
            nc.gpsimd.load_library(library_config.attn)

            xt_sb = [[const.tile([P, blkw], MM_DT, name=f"xt{b}_{k}")
                      for k in range(KCH)] for b in range(nblk)]
            rhs_sb = [const.tile([P, c_pad], MM_DT, name=f"rhs{k}")
                      for k in range(KCH)]
            m2bc_sb = const.tile([P, c_pad], F32, name="m2bc_sb")

            # startup-latency-aware order: tile 0 only needs the first P
            # columns of each xt[0][k] plus the rhs chunks, in k order; the
            # first matmul gates on ~160KB. Everything else streams behind.
            nc.sync.dma_start(rhs_sb[0][:, 0:NB], rhs[0][:, 0:NB])
            nc.sync.dma_start(xt_sb[0][0][:, 0:P], xt[0, 0][:, 0:P])
            nc.sync.dma_start(rhs_sb[0][:, NB:], rhs[0][:, NB:])
            nc.sync.dma_start(xt_sb[0][1][:, 0:P], xt[0, 1][:, 0:P])
            nc.sync.dma_start(rhs_sb[1][:], rhs[1])
            nc.sync.dma_start(xt_sb[0][2][:, 0:P], xt[0, 2][:, 0:P])
            nc.sync.dma_start(rhs_sb[2][:], rhs[2])
            nc.sync.dma_start(xt_sb[0][3][:, 0:P], xt[0, 3][:, 0:P])
            nc.sync.dma_start(rhs_sb[3][:], rhs[3])
            # DMA engines drain one FIFO, so order by first use: block-0
            # remainders (tiles 1-7) ahead of m2bc (first needed by tile
            # 0's sub+max, ~2.4us after its matmuls); block 1 last.
            # Blocks 2-3 are enqueued inside the tile loop so the first
            # tiles' output transfers aren't queued behind them.
            nc.sync.dma_start(xt_sb[0][0][:, P:], xt[0, 0][:, P:])
            nc.sync.dma_start(xt_sb[0][1][:, P:], xt[0, 1][:, P:])
            nc.sync.dma_start(m2bc_sb[:], m2bc[:])
            nc.sync.dma_start(xt_sb[0][2][:, P:], xt[0, 2][:, P:])
            nc.sync.dma_start(xt_sb[0][3][:, P:], xt[0, 3][:, P:])
            for k in range(KCH):
                nc.sync.dma_start(xt_sb[1][k][:], xt[1, k])

            for i in range(ntiles):
                if i == 2 and nblk > 2:
                    for k in range(KCH):
                        nc.sync.dma_start(xt_sb[2][k][:], xt[2, k])
                if i == 5 and nblk > 3:
                    for k in range(KCH):
                        nc.sync.dma_start(xt_sb[3][k][:], xt[3, k])
                last = (i == ntiles - 1)
                ps = psum.tile([P, psum_w], F32)
                blk, off = divmod(i, tpb)
                lhsT = [xt_sb[blk][k][:, off * P:(off + 1) * P]
                        for k in range(KCH)]
                sco = ssp.tile([P, c_pad], F32)
                ot = outp.tile([P, c_pad], F32)
                zs = stat.tile([P, 1], F32)
                nm = stat.tile([P, 1], F32)
                if not last:
                    for k in range(KCH):
                        for c0, w in chunks:
                            nc.tensor.matmul(
                                ps[:, c0:c0 + w], lhsT[k],
                                rhs_sb[k][:, c0:c0 + w],
                                start=(k == 0), stop=(k == KCH - 1),
                            )
                    # sco = 2 x.mu - m2c = scores; mx = rowmax (one DVE pass)
                    mx = stat.tile([P, 1], F32)
                    nc.vector._custom_dve(
                        NCM_SUB_MAX, out=sco[:], accum_out=mx[:],
                        in0=ps[:, 0:c_pad], in1=m2bc_sb[:],
                    )
                    nc.vector.tensor_scalar_mul(nm[:], mx[:], -1.0)
                    # ot = exp(sco - max); zs = sum(ot)
                    nc.scalar.activation(
                        ot[:], sco[:], AF.Exp,
                        bias=nm[:], accum_out=zs[:],
                    )
                    # normalize entirely on the (otherwise idle) GpSimd;
                    # normalize_recip divides by raw zs and clobbers it
                    # with 1/zs, which we don't reuse
                    nc.gpsimd.normalize_recip(ot[:], ot[:], zs[:])
                    nc.sync.dma_start(out[i * P:(i + 1) * P, :], ot[:])
                else:
                    # drain-optimized last tile: c-major matmuls so each
                    # column chunk's sub+max runs under the next chunk's
                    # matmuls; only the final chunk's reduction remains
                    # after the last matmul
                    mxs = stat.tile([P, len(chunks)], F32)
                    for ci, (c0, w) in enumerate(chunks):
                        for k in range(KCH):
                            nc.tensor.matmul(
                                ps[:, c0:c0 + w], lhsT[k],
                                rhs_sb[k][:, c0:c0 + w],
                                start=(k == 0), stop=(k == KCH - 1),
                            )
                        nc.vector._custom_dve(
                            NCM_SUB_MAX, out=sco[:, c0:c0 + w],
                            accum_out=mxs[:, ci:ci + 1],
                            in0=ps[:, c0:c0 + w],
                            in1=m2bc_sb[:, c0:c0 + w],
                        )
                    nc.vector.tensor_reduce(
                        nm[:], mxs[:], mybir.AxisListType.X,
                        mybir.AluOpType.max, negate=True,
                    )
                    nc.scalar.activation(
                        ot[:], sco[:], AF.Exp,
                        bias=nm[:], accum_out=zs[:],
                    )
                    # normalize+ship in halves: DVE and GpSimd in parallel
                    h = c_pad // 2
                    rz = stat.tile([P, 1], F32)
                    nc.vector.reciprocal(rz[:], zs[:])
                    nc.vector.tensor_scalar_mul(ot[:, :h], ot[:, :h], rz[:])
                    nc.gpsimd.normalize_recip(ot[:, h:], ot[:, h:], zs[:])
                    nc.sync.dma_start(out[i * P:(i + 1) * P, :h], ot[:, :h])
                    nc.sync.dma_start(out[i * P:(i + 1) * P, h:], ot[:, h:])

    nc.compile()
    return nc


_NC_CACHE = {}


def _get_nc(c_pad):
    if c_pad not in _NC_CACHE:
        _NC_CACHE[c_pad] = build_nc(c_pad)
    return _NC_CACHE[c_pad]


def prep_inputs(X, muK, cK, c_pad, keep):
    """Host-side shard/layout prep (numpy only)."""
    X = np.asarray(X, dtype=np.float32)
    muK = np.asarray(muK, dtype=np.float32)
    nk = len(keep)

    muKk = muK[keep]                                    # [nk, D]
    m2 = np.sum(muKk.astype(np.float64) ** 2, axis=1)
    m2c = m2 - m2.mean()  # centered: softmax-invariant shift
    m2p = np.full(c_pad, MASK_M2, dtype=np.float32)
    m2p[:nk] = m2c.astype(np.float32)
    m2bc_np = np.ascontiguousarray(np.broadcast_to(m2p[None, :], (P, c_pad)))

    rhsw = np.zeros((D, c_pad), dtype=np.float16)
    rhsw[:, :nk] = (2.0 * muKk.T).astype(np.float16)
    rhs_np = np.ascontiguousarray(rhsw.reshape(KCH, P, c_pad))

    Xt = X.T.astype(np.float16)  # [D, N]
    ntiles = NS // P
    nblk = max(1, ntiles // TILES_PER_BLK)
    blkw = NS // nblk
    in_maps = []
    for core in range(NCORES):
        xs = Xt[:, core * NS:(core + 1) * NS]              # [D, NS]
        xs = xs.reshape(KCH, P, nblk, blkw).transpose(2, 0, 1, 3)
        in_maps.append({"xt": np.ascontiguousarray(xs),
                        "rhs": rhs_np, "m2bc": m2bc_np})
    return in_maps


def run(X, muK, cK, trace=False, **kw):
    cK = np.asarray(cK, dtype=np.float32)
    keep = np.nonzero(cK != 0.0)[0]
    nk = len(keep)
    if nk == 0:
        # every class masked -> all scores equal -> uniform softmax
        return np.full((N, C), 1.0 / C, dtype=np.float32), None
    c_pad = C_PAD if nk <= C_PAD else C
    in_maps = prep_inputs(X, muK, cK, c_pad, keep)
    nc = _get_nc(c_pad)
    res = run_bass_kernel_spmd(
        nc, in_maps, list(range(NCORES)), trace=trace, **kw)
    compact = np.concatenate(
        [res.results[c]["out"] for c in range(NCORES)], axis=0)
    full = np.zeros((N, C), dtype=np.float32)
    full[:, keep] = compact[:, :nk]
    return full, res


def kernel(X, muK, cK):
    full, _ = run(X, muK, cK, trace=False)
    return full
